# revision 3
# baseline (speedup 1.0000x reference)
"""Trainium2 Bass kernel for nn_BigramLMLinear (embedding lookup).

Math: out[b, t, :] = W[:, inputs[b, t]]  ==  W.T[inputs[b, t], :]
  W: [16384, 16384] f32, inputs: [8, 2048] int, out: [8, 2048, 16384] f32.

Strategy: data-parallel over the 8*2048 tokens — each of the 8 NeuronCores
handles one batch row (2048 tokens) and holds a full replica of WT = W.T
(pre-transposed on the host so each lookup is a contiguous 64 KiB row read).
No collectives.

Two device-kernel variants (MODE):
  "bounce": all 2048 rows via gpsimd indirect-gather HBM->SBUF (128 rows /
      8 MiB per tile) + HWDGE store SBUF->HBM, Tile-scheduled. Bit-exact,
      measured 815 us on HW (SDMA-engine-bound: every byte crosses the
      engines twice).
  "hybrid": R rows go DRAM->DRAM directly via register-offset HWDGE DMAs
      on sync+scalar (1x engine bytes, ~1.4 us/row issue), the rest via the
      bounce path on gpsimd (2x engine bytes, cheap issue), raw-Bass
      scheduled.
"""

import numpy as np

V = 16384          # vocab (rows of WT) and embedding dim (cols)
B = 8              # batch rows == number of cores
T = 2048           # tokens per core
P = 128            # SBUF partitions
ROW_B = V * 4      # bytes per row (64 KiB)
N_CORES = 8

MODE = "bounce"    # "bounce" | "hybrid"
HYB_R = 896        # hybrid: rows via register-offset D2D (multiple of 256)

_CACHE = {}
LAST_RESULTS = None  # BassKernelResults of the most recent run (for test harness)


def _build_bounce():
    import concourse.bacc as bacc
    import concourse.bass as bass
    import concourse.mybir as mybir
    import concourse.tile as tile

    n_tiles = T // P
    nc = bacc.Bacc("TRN2", target_bir_lowering=False, debug=False)
    ids_ext = nc.declare_dram_parameter("ids", [P, n_tiles], mybir.dt.int32, isOutput=False)
    wt_ext = nc.declare_dram_parameter("wt", [V, V], mybir.dt.float32, isOutput=False)
    out_ext = nc.declare_dram_parameter("out", [T, V], mybir.dt.float32, isOutput=True)

    with tile.TileContext(nc) as tc:
        with (
            tc.tile_pool(name="idp", bufs=1) as idpool,
            tc.tile_pool(name="rows", bufs=2) as pool,
        ):
            ids_sb = idpool.tile([P, n_tiles], mybir.dt.int32)
            nc.sync.dma_start(out=ids_sb[:], in_=ids_ext[:])
            for t in range(n_tiles):
                row_tile = pool.tile([P, V], mybir.dt.float32)
                nc.gpsimd.indirect_dma_start(
                    out=row_tile[:],
                    out_offset=None,
                    in_=wt_ext[:],
                    in_offset=bass.IndirectOffsetOnAxis(ap=ids_sb[:, t : t + 1], axis=0),
                )
                nc.sync.dma_start(out=out_ext[t * P : (t + 1) * P, :], in_=row_tile[:])
    nc.compile()
    return nc


def _build_hybrid(R):
    import concourse.bass as bass
    import concourse.mybir as mybir

    Bn = T - R
    nb_tiles = Bn // P
    nc = bass.Bass("TRN2", target_bir_lowering=False, debug=False)
    ids_r_ext = nc.dram_tensor("ids_r", [1, R], mybir.dt.int32, kind="ExternalInput")
    ids_g_ext = nc.dram_tensor("ids_g", [P, nb_tiles], mybir.dt.int32, kind="ExternalInput")
    wt_ext = nc.dram_tensor("wt", [1, V * ROW_B], mybir.dt.uint8, kind="ExternalInput")
    out_ext = nc.dram_tensor("out", [T, ROW_B], mybir.dt.uint8, kind="ExternalOutput")

    ids_r_sb = nc.alloc_sbuf_tensor("ids_r_sb", [1, R], mybir.dt.int32)
    ids_g_sb = nc.alloc_sbuf_tensor("ids_g_sb", [P, nb_tiles], mybir.dt.int32)
    bufs = [
        nc.alloc_sbuf_tensor("buf0", [P, ROW_B], mybir.dt.uint8),
        nc.alloc_sbuf_tensor("buf1", [P, ROW_B], mybir.dt.uint8),
    ]
    wt2d = wt_ext[:].rearrange("1 (v d) -> v d", d=ROW_B)
    half = R // 2
    MAXOFF = (V - 1) * ROW_B

    with (
        nc.semaphore("ids_sem") as ids_sem,
        nc.semaphore("sp_sem") as sp_sem,
        nc.semaphore("act_sem") as act_sem,
        nc.semaphore("g_sem") as g_sem,
        nc.semaphore("st_sem") as st_sem,
        nc.Block() as block,
    ):

        @block.sync
        def _(sync):
            sync.dma_start(out=ids_r_sb[:], in_=ids_r_ext[:]).then_inc(ids_sem, 16)
            sync.dma_start(out=ids_g_sb[:], in_=ids_g_ext[:]).then_inc(ids_sem, 16)
            sync.wait_ge(ids_sem, 32)
            with sync.register("rsp") as rid:
                for i in range(half):
                    sync.reg_load(rid, ids_r_sb[0:1, i : i + 1])
                    off = bass.make_scalar_value(
                        bass.RegisterHandles(rid), min_val=0, max_val=MAXOFF
                    )
                    sync.dma_start(
                        out=out_ext[i : i + 1, :],
                        in_=wt_ext[0:1, bass.ds(off, ROW_B)],
                    ).then_inc(sp_sem, 16)
            sync.wait_ge(sp_sem, 16 * half)

        @block.scalar
        def _(scalar):
            scalar.wait_ge(ids_sem, 32)
            with scalar.register("ract") as rid:
                for i in range(half, R):
                    scalar.reg_load(rid, ids_r_sb[0:1, i : i + 1])
                    off = bass.make_scalar_value(
                        bass.RegisterHandles(rid), min_val=0, max_val=MAXOFF
                    )
                    scalar.dma_start(
                        out=out_ext[i : i + 1, :],
                        in_=wt_ext[0:1, bass.ds(off, ROW_B)],
                    ).then_inc(act_sem, 16)
            scalar.wait_ge(act_sem, 16 * (R - half))

        @block.gpsimd
        def _(gpsimd):
            gpsimd.wait_ge(ids_sem, 32)

            def emit_gather(t):
                gpsimd.indirect_dma_start(
                    out=bufs[t % 2][:],
                    out_offset=None,
                    in_=wt2d,
                    in_offset=bass.IndirectOffsetOnAxis(
                        ap=ids_g_sb[:, t : t + 1], axis=0
                    ),
                ).then_inc(g_sem, 16)

            emit_gather(0)
            for t in range(nb_tiles):
                if t + 1 < nb_tiles:
                    if t >= 1:
                        gpsimd.wait_ge(st_sem, 16 * t)
                    emit_gather(t + 1)
                gpsimd.wait_ge(g_sem, 16 * (t + 1))
                gpsimd.dma_start(
                    out=out_ext[R + t * P : R + (t + 1) * P, :], in_=bufs[t % 2][:]
                ).then_inc(st_sem, 16)
            gpsimd.wait_ge(st_sem, 16 * nb_tiles)

    return nc


def _ensure_axon_hooks_importable():
    """bass_utils imports antenv.axon_hooks when tracing is requested
    (BASS_TRACE). Some images ship an antenv stub without it; register a
    no-op hook module so tracing degrades gracefully instead of crashing.
    No-op when the real module exists."""
    try:
        import antenv.axon_hooks  # noqa: F401
    except ImportError:
        import sys
        import types

        try:
            import antenv
        except ImportError:
            return
        mod = types.ModuleType("antenv.axon_hooks")
        state = {"h": None}
        mod.set_axon_ntff_profile_hook = lambda h: state.__setitem__("h", h)
        mod.get_axon_ntff_profile_hook = lambda: state["h"]
        sys.modules["antenv.axon_hooks"] = mod
        antenv.axon_hooks = mod


def kernel(inputs: np.ndarray, W: np.ndarray) -> np.ndarray:
    global LAST_RESULTS
    _ensure_axon_hooks_importable()
    from concourse.bass_utils import run_bass_kernel_spmd

    ids = np.asarray(inputs).astype(np.int32)           # [B, T]
    assert ids.shape == (B, T)
    wt = np.ascontiguousarray(np.asarray(W).T)          # [V, V]; row i == W[:, i]

    if MODE == "bounce":
        if "nc_b" not in _CACHE:
            _CACHE["nc_b"] = _build_bounce()
        nc = _CACHE["nc_b"]
        n_tiles = T // P
        in_maps = []
        for c in range(N_CORES):
            # column t of the [P, n_tiles] layout = tokens t*P .. t*P+127
            ids_c = np.ascontiguousarray(ids[c].reshape(n_tiles, P).T)
            in_maps.append({"ids": ids_c, "wt": wt})
        res = run_bass_kernel_spmd(nc, in_maps, core_ids=list(range(N_CORES)))
        LAST_RESULTS = res
        return np.stack([res.results[c]["out"] for c in range(N_CORES)], axis=0)

    # hybrid
    R = HYB_R
    nb_tiles = (T - R) // P
    if "nc_h" not in _CACHE:
        _CACHE["nc_h"] = _build_hybrid(R)
    nc = _CACHE["nc_h"]
    wt_u8 = wt.reshape(1, -1).view(np.uint8)
    in_maps = []
    for c in range(N_CORES):
        ids_c = ids[c]
        in_maps.append({
            "ids_r": (ids_c[:R].astype(np.int64) * ROW_B).astype(np.int32)[None, :],
            "ids_g": np.ascontiguousarray(ids_c[R:].reshape(nb_tiles, P).T),
            "wt": wt_u8,
        })
    res = run_bass_kernel_spmd(nc, in_maps, core_ids=list(range(N_CORES)))
    LAST_RESULTS = res
    out = np.stack(
        [res.results[c]["out"].reshape(T, ROW_B) for c in range(N_CORES)], axis=0
    )
    return out.view(np.float32)  # [B, T, V]


# revision 6
# speedup vs baseline: 1.1132x; 1.1132x over previous
"""Trainium2 Bass kernel for nn_BigramLMLinear (embedding lookup).

Math: out[b, t, :] = W[:, inputs[b, t]]  ==  W.T[inputs[b, t], :]
  W: [16384, 16384] f32, inputs: [8, 2048] int, out: [8, 2048, 16384] f32.

Strategy: data-parallel over the 8*2048 tokens — each of the 8 NeuronCores
handles one batch row (2048 tokens) and holds a full replica of WT = W.T
(pre-transposed on the host so each lookup is a contiguous 64 KiB row read).
No collectives.

Two device-kernel variants (MODE):
  "bounce": all 2048 rows via gpsimd indirect-gather HBM->SBUF (128 rows /
      8 MiB per tile) + HWDGE store SBUF->HBM, Tile-scheduled. Bit-exact,
      measured 815 us on HW (SDMA-engine-bound: every byte crosses the
      engines twice).
  "hybrid": R rows go DRAM->DRAM directly via register-offset HWDGE DMAs
      on sync+scalar (1x engine bytes, ~1.4 us/row issue), the rest via the
      bounce path on gpsimd (2x engine bytes, cheap issue), raw-Bass
      scheduled.
"""

import numpy as np

V = 16384          # vocab (rows of WT) and embedding dim (cols)
B = 8              # batch rows == number of cores
T = 2048           # tokens per core
P = 128            # SBUF partitions
ROW_B = V * 4      # bytes per row (64 KiB)
N_CORES = 8

import os as _os

MODE = _os.environ.get("BLM_MODE", "bounce")   # "bounce" | "bounce2" | "hybrid"
HYB_R = 896        # hybrid: rows via register-offset D2D (multiple of 256)

_CACHE = {}
LAST_RESULTS = None  # BassKernelResults of the most recent run (for test harness)


def _build_bounce():
    import concourse.bacc as bacc
    import concourse.bass as bass
    import concourse.mybir as mybir
    import concourse.tile as tile

    n_tiles = T // P
    nc = bacc.Bacc("TRN2", target_bir_lowering=False, debug=False)
    ids_ext = nc.declare_dram_parameter("ids", [P, n_tiles], mybir.dt.int32, isOutput=False)
    wt_ext = nc.declare_dram_parameter("wt", [V, V], mybir.dt.float32, isOutput=False)
    out_ext = nc.declare_dram_parameter("out", [T, V], mybir.dt.float32, isOutput=True)

    with tile.TileContext(nc) as tc:
        with (
            tc.tile_pool(name="idp", bufs=1) as idpool,
            tc.tile_pool(name="rows", bufs=2) as pool,
        ):
            ids_sb = idpool.tile([P, n_tiles], mybir.dt.int32)
            nc.sync.dma_start(out=ids_sb[:], in_=ids_ext[:])
            for t in range(n_tiles):
                row_tile = pool.tile([P, V], mybir.dt.float32)
                nc.gpsimd.indirect_dma_start(
                    out=row_tile[:],
                    out_offset=None,
                    in_=wt_ext[:],
                    in_offset=bass.IndirectOffsetOnAxis(ap=ids_sb[:, t : t + 1], axis=0),
                )
                nc.sync.dma_start(out=out_ext[t * P : (t + 1) * P, :], in_=row_tile[:])
    nc.compile()
    return nc


def _build_bounce2():
    """Half-row variant: gather 32 KiB half-rows from WT viewed as
    [2V, V/2] (token halves = rows 2id, 2id+1), 4 SBUF buffers, stores
    alternating across both HWDGE rings (sync/scalar)."""
    import concourse.bacc as bacc
    import concourse.bass as bass
    import concourse.mybir as mybir
    import concourse.tile as tile

    H = V // 2
    n_tiles = T // P
    nc = bacc.Bacc("TRN2", target_bir_lowering=False, debug=False)
    ids_ext = nc.declare_dram_parameter("ids2", [P, 2 * n_tiles], mybir.dt.int32, isOutput=False)
    wt_ext = nc.declare_dram_parameter("wt", [2 * V, H], mybir.dt.float32, isOutput=False)
    out_ext = nc.declare_dram_parameter("out", [T, V], mybir.dt.float32, isOutput=True)

    with tile.TileContext(nc) as tc:
        with (
            tc.tile_pool(name="idp", bufs=1) as idpool,
            tc.tile_pool(name="rows", bufs=4) as pool,
        ):
            ids_sb = idpool.tile([P, 2 * n_tiles], mybir.dt.int32)
            nc.sync.dma_start(out=ids_sb[:], in_=ids_ext[:])
            for t in range(n_tiles):
                for h in range(2):
                    col = 2 * t + h
                    row_tile = pool.tile([P, H], mybir.dt.float32)
                    nc.gpsimd.indirect_dma_start(
                        out=row_tile[:],
                        out_offset=None,
                        in_=wt_ext[:],
                        in_offset=bass.IndirectOffsetOnAxis(
                            ap=ids_sb[:, col : col + 1], axis=0
                        ),
                    )
                    eng = nc.sync if col % 2 == 0 else nc.scalar
                    eng.dma_start(
                        out=out_ext[t * P : (t + 1) * P, h * H : (h + 1) * H],
                        in_=row_tile[:],
                    )
    nc.compile()
    return nc


def _build_hybrid(R):
    import concourse.bass as bass
    import concourse.mybir as mybir

    Bn = T - R
    nb_tiles = Bn // P
    nc = bass.Bass("TRN2", target_bir_lowering=False, debug=False)
    ids_r_ext = nc.dram_tensor("ids_r", [1, R], mybir.dt.int32, kind="ExternalInput")
    ids_g_ext = nc.dram_tensor("ids_g", [P, nb_tiles], mybir.dt.int32, kind="ExternalInput")
    wt_ext = nc.dram_tensor("wt", [1, V * ROW_B], mybir.dt.uint8, kind="ExternalInput")
    out_ext = nc.dram_tensor("out", [T, ROW_B], mybir.dt.uint8, kind="ExternalOutput")

    ids_r_sb = nc.alloc_sbuf_tensor("ids_r_sb", [1, R], mybir.dt.int32)
    ids_g_sb = nc.alloc_sbuf_tensor("ids_g_sb", [P, nb_tiles], mybir.dt.int32)
    bufs = [
        nc.alloc_sbuf_tensor("buf0", [P, ROW_B], mybir.dt.uint8),
        nc.alloc_sbuf_tensor("buf1", [P, ROW_B], mybir.dt.uint8),
    ]
    wt2d = wt_ext[:].rearrange("1 (v d) -> v d", d=ROW_B)
    half = R // 2
    MAXOFF = (V - 1) * ROW_B

    with (
        nc.semaphore("ids_sem") as ids_sem,
        nc.semaphore("sp_sem") as sp_sem,
        nc.semaphore("act_sem") as act_sem,
        nc.semaphore("g_sem") as g_sem,
        nc.semaphore("st_sem") as st_sem,
        nc.Block() as block,
    ):

        @block.sync
        def _(sync):
            sync.dma_start(out=ids_r_sb[:], in_=ids_r_ext[:]).then_inc(ids_sem, 16)
            sync.dma_start(out=ids_g_sb[:], in_=ids_g_ext[:]).then_inc(ids_sem, 16)
            sync.wait_ge(ids_sem, 32)
            with sync.register("rsp") as rid:
                for i in range(half):
                    sync.reg_load(rid, ids_r_sb[0:1, i : i + 1])
                    off = bass.make_scalar_value(
                        bass.RegisterHandles(rid), min_val=0, max_val=MAXOFF
                    )
                    sync.dma_start(
                        out=out_ext[i : i + 1, :],
                        in_=wt_ext[0:1, bass.ds(off, ROW_B)],
                    ).then_inc(sp_sem, 16)
            sync.wait_ge(sp_sem, 16 * half)

        @block.scalar
        def _(scalar):
            scalar.wait_ge(ids_sem, 32)
            with scalar.register("ract") as rid:
                for i in range(half, R):
                    scalar.reg_load(rid, ids_r_sb[0:1, i : i + 1])
                    off = bass.make_scalar_value(
                        bass.RegisterHandles(rid), min_val=0, max_val=MAXOFF
                    )
                    scalar.dma_start(
                        out=out_ext[i : i + 1, :],
                        in_=wt_ext[0:1, bass.ds(off, ROW_B)],
                    ).then_inc(act_sem, 16)
            scalar.wait_ge(act_sem, 16 * (R - half))

        @block.gpsimd
        def _(gpsimd):
            gpsimd.wait_ge(ids_sem, 32)

            def emit_gather(t):
                gpsimd.indirect_dma_start(
                    out=bufs[t % 2][:],
                    out_offset=None,
                    in_=wt2d,
                    in_offset=bass.IndirectOffsetOnAxis(
                        ap=ids_g_sb[:, t : t + 1], axis=0
                    ),
                ).then_inc(g_sem, 16)

            emit_gather(0)
            for t in range(nb_tiles):
                if t + 1 < nb_tiles:
                    if t >= 1:
                        gpsimd.wait_ge(st_sem, 16 * t)
                    emit_gather(t + 1)
                gpsimd.wait_ge(g_sem, 16 * (t + 1))
                gpsimd.dma_start(
                    out=out_ext[R + t * P : R + (t + 1) * P, :], in_=bufs[t % 2][:]
                ).then_inc(st_sem, 16)
            gpsimd.wait_ge(st_sem, 16 * nb_tiles)

    return nc


def _ensure_axon_hooks_importable():
    """bass_utils imports antenv.axon_hooks when tracing is requested
    (BASS_TRACE). Some images ship an antenv stub without it; register a
    no-op hook module so tracing degrades gracefully instead of crashing.
    No-op when the real module exists."""
    try:
        import antenv.axon_hooks  # noqa: F401
    except ImportError:
        import sys
        import types

        try:
            import antenv
        except ImportError:
            return
        mod = types.ModuleType("antenv.axon_hooks")
        state = {"h": None}
        mod.set_axon_ntff_profile_hook = lambda h: state.__setitem__("h", h)
        mod.get_axon_ntff_profile_hook = lambda: state["h"]
        sys.modules["antenv.axon_hooks"] = mod
        antenv.axon_hooks = mod


def kernel(inputs: np.ndarray, W: np.ndarray) -> np.ndarray:
    global LAST_RESULTS
    _ensure_axon_hooks_importable()
    from concourse.bass_utils import run_bass_kernel_spmd

    ids = np.asarray(inputs).astype(np.int32)           # [B, T]
    assert ids.shape == (B, T)
    wt = np.ascontiguousarray(np.asarray(W).T)          # [V, V]; row i == W[:, i]

    if MODE == "bounce":
        if "nc_b" not in _CACHE:
            _CACHE["nc_b"] = _build_bounce()
        nc = _CACHE["nc_b"]
        n_tiles = T // P
        in_maps = []
        for c in range(N_CORES):
            # column t of the [P, n_tiles] layout = tokens t*P .. t*P+127
            ids_c = np.ascontiguousarray(ids[c].reshape(n_tiles, P).T)
            in_maps.append({"ids": ids_c, "wt": wt})
        res = run_bass_kernel_spmd(nc, in_maps, core_ids=list(range(N_CORES)))
        LAST_RESULTS = res
        return np.stack([res.results[c]["out"] for c in range(N_CORES)], axis=0)

    if MODE == "bounce2":
        if "nc_b2" not in _CACHE:
            _CACHE["nc_b2"] = _build_bounce2()
        nc = _CACHE["nc_b2"]
        n_tiles = T // P
        wt2 = wt.reshape(2 * V, V // 2)
        in_maps = []
        for c in range(N_CORES):
            ids_t = ids[c].reshape(n_tiles, P)  # [t, p]
            ids2 = np.empty((P, 2 * n_tiles), np.int32)
            ids2[:, 0::2] = (2 * ids_t).T
            ids2[:, 1::2] = (2 * ids_t + 1).T
            in_maps.append({"ids2": ids2, "wt": wt2})
        res = run_bass_kernel_spmd(nc, in_maps, core_ids=list(range(N_CORES)))
        LAST_RESULTS = res
        return np.stack([res.results[c]["out"] for c in range(N_CORES)], axis=0)

    # hybrid
    R = HYB_R
    nb_tiles = (T - R) // P
    if "nc_h" not in _CACHE:
        _CACHE["nc_h"] = _build_hybrid(R)
    nc = _CACHE["nc_h"]
    wt_u8 = wt.reshape(1, -1).view(np.uint8)
    in_maps = []
    for c in range(N_CORES):
        ids_c = ids[c]
        in_maps.append({
            "ids_r": (ids_c[:R].astype(np.int64) * ROW_B).astype(np.int32)[None, :],
            "ids_g": np.ascontiguousarray(ids_c[R:].reshape(nb_tiles, P).T),
            "wt": wt_u8,
        })
    res = run_bass_kernel_spmd(nc, in_maps, core_ids=list(range(N_CORES)))
    LAST_RESULTS = res
    out = np.stack(
        [res.results[c]["out"].reshape(T, ROW_B) for c in range(N_CORES)], axis=0
    )
    return out.view(np.float32)  # [B, T, V]
